# revision 24
# baseline (speedup 1.0000x reference)
"""Trainium2 Bass kernel for ragged multi-head attention (B=16,S=512,H=1024,NH=16).

Sharding: data-parallel over batch — 16 samples over 8 cores, 2 samples/core.
Per core, everything is computed in a "transposed activation" layout so no
on-chip transposes are ever needed:

  - host passes x pre-transposed per core:        xT   [H, T=1024]  (tokens of 2 samples)
  - host passes weights pre-transposed:           w*T  [H_in, H_out]
  - Q^T, K^T projections produce [H_out, T]       (partition = head dim!)
  - scores^T[k,q] = (K^T_h chunk).T @ Q^T_h       (contraction d=64 on partitions;
    odd/even heads sit at partition offsets 0/64, so a head-pair's score
    matmuls run concurrently in disjoint PE row groups)
  - softmax without max-subtraction (scores are O(1)); the key mask folds into
    the exp's per-partition bias; row-sums come free from a ones column
    appended to V ("V_aug"), computed by the same PV matmul
  - out^T_h[d,q] = V_aug.T @ P^T ; row 64 = softmax denominators
  - normalization: reciprocal (DVE) -> broadcast across 64 partitions
    (GPSIMD partition_broadcast) -> multiply (DVE)
  - fc output is computed in natural [t, o] layout (attnout^T chunks are the
    stationary operand), so the result DMAs out contiguously.

Ragged: per-sample lengths known on host; samples sorted by length into 2
"slots" (slot n-of-tiles = max over the 8 cores' samples in that slot), and all
per-token loops only cover ceil(L/128) 128-token tiles. Rows at/after cls_len
are exactly `bf` in the reference and are filled on the host.
"""

import math

import numpy as np

B, S, H, NH, DH = 16, 512, 1024, 16, 64
N_CORES = 8
B_LOC = B // N_CORES  # 2
T = B_LOC * S  # 1024 tokens per core
P = 128
G = H // P  # 8 contraction chunks
NEG = -30.0  # additive bias for masked keys: exp(-30) ~ 9e-14

_CACHE = {}

import os as _os
_OLD_TAIL = bool(int(_os.environ.get("K_OLD_TAIL", "0")))


def _build(ns, loop_iters=None, dtcfg=None, sections=None):
    """Build the Bass program for per-slot tile counts ns=(n0,n1).

    loop_iters: if not None, wrap the whole body in a hardware For_i loop
    (used only for benchmarking).  Returns the compiled nc.
    sections: None for the full kernel, else a set of section names from
    {"qk", "v", "attn", "fc"} to include (benchmark-only).
    """
    import concourse.bass as bass
    import concourse.mybir as mybir
    import concourse.tile as tile
    from concourse import bacc

    dt = mybir.dt
    cfg = dict(
        qk=dt.bfloat16,   # xT / wq / wk / wv / Q^T / K^T storage + matmul dtype
        pv=dt.bfloat16,   # P^T / V_aug storage + PV matmul dtype
        fc=dt.bfloat16,   # attnout^T / wfT storage + FC matmul dtype
    )
    if dtcfg:
        cfg.update(dtcfg)
    FQK = cfg["qk"]
    FPV = cfg["pv"]
    FFC = cfg["fc"]

    nc = bacc.Bacc("TRN2", target_bir_lowering=False, debug=False,
                   num_devices=N_CORES)

    f32 = dt.float32
    f32r = dt.float32r
    xT = nc.dram_tensor("xT", [H, T], FQK, kind="ExternalInput")
    # wq/wk host-swizzled to [j, p, g, c]: column-block j is one contiguous
    # 256KB read with 2KB runs (strided reads of 256B pay a 2x DMA penalty)
    wqT = nc.dram_tensor("wqT", [G, P, G, P], FQK, kind="ExternalInput")
    wkT = nc.dram_tensor("wkT", [G, P, G, P], FQK, kind="ExternalInput")
    wvT = nc.dram_tensor("wvT", [H, H], FQK, kind="ExternalInput")
    wfT = nc.dram_tensor("wfT", [H, H], FFC, kind="ExternalInput")
    bq2 = nc.dram_tensor("bq2", [G, P], f32, kind="ExternalInput")
    bk2 = nc.dram_tensor("bk2", [G, P], f32, kind="ExternalInput")
    bv1 = nc.dram_tensor("bv1", [1, H], f32, kind="ExternalInput")
    bf1 = nc.dram_tensor("bf1", [1, H], f32, kind="ExternalInput")
    kbias = nc.dram_tensor("kbias", [B_LOC, 4, P], f32, kind="ExternalInput")
    y = nc.dram_tensor("y", [T, H], f32, kind="ExternalOutput")

    HA = 128  # per-head V columns: 0:64 = V, 64:128 = ones (den replicas)

    with tile.TileContext(nc) as tc:
        import contextlib
        ctx = contextlib.ExitStack()
        with ctx:
            const = ctx.enter_context(tc.tile_pool(name="const", bufs=1))
            wqk_pool = ctx.enter_context(tc.tile_pool(name="wqk", bufs=3))
            pt_pool = ctx.enter_context(tc.tile_pool(name="pt", bufs=18))
            rb_pool = ctx.enter_context(tc.tile_pool(name="rb", bufs=4))
            out_pool = ctx.enter_context(tc.tile_pool(name="out", bufs=4))
            ps_mm = ctx.enter_context(tc.tile_pool(name="psmm", bufs=3, space="PSUM"))
            ps_sc = ctx.enter_context(tc.tile_pool(name="pssc", bufs=3, space="PSUM"))
            ps_pv = ctx.enter_context(tc.tile_pool(name="pspv", bufs=2, space="PSUM"))

            # resident tensors
            xT_sb = const.tile([P, G, T], FQK)
            qT_sb = const.tile([P, G, T], FQK)
            # K^T stored zero-padded so score matmuls contract over the full
            # 128 partitions (64-row matmuls run at half the column rate):
            # kTe rows 0:64 = even head, rows 64:128 = 0; kTo the reverse.
            kTe_sb = const.tile([P, G, T], FQK)
            kTo_sb = const.tile([P, G, T], FQK)
            vaug_sb = const.tile([P, 2 * 4, NH * HA], FPV)
            aoT_sb = const.tile([P, G, T], FFC)
            wv_sb = const.tile([P, G, H], FQK)
            wf_sb = const.tile([P, G, H], FFC)
            bq_sb = const.tile([P, G], f32)
            bk_sb = const.tile([P, G], f32)
            bv_sb = const.tile([P, H], f32)
            bf_sb = const.tile([P, H], f32)
            kb_sb = const.tile([P, B_LOC, 4], f32)

            def body():
                # ---- preload ----
                # xT chunks alternate between the SP and ACT HWDGE queues so
                # the first compute-gating load finishes sooner; everything
                # else the first matmuls don't need goes on the ACT queue.
                for g in range(G):
                    eng = nc.sync if g % 2 == 0 else nc.scalar
                    eng.dma_start(
                        out=xT_sb[:, g, :],
                        in_=xT.ap()[g * P:(g + 1) * P, :])
                nc.scalar.dma_start(
                    out=bq_sb[:], in_=bq2.ap().rearrange("g p -> p g"))
                nc.scalar.dma_start(
                    out=bk_sb[:], in_=bk2.ap().rearrange("g p -> p g"))
                bvap = bv1.ap()
                nc.scalar.dma_start(
                    out=bv_sb[:],
                    in_=bass.AP(tensor=bvap.tensor, offset=bvap.offset,
                                ap=[[0, P]] + list(bvap.ap[1:])))
                bfap = bf1.ap()
                nc.scalar.dma_start(
                    out=bf_sb[:],
                    in_=bass.AP(tensor=bfap.tensor, offset=bfap.offset,
                                ap=[[0, P]] + list(bfap.ap[1:])))
                nc.scalar.dma_start(
                    out=kb_sb[:], in_=kbias.ap().rearrange("s k p -> p s k"))
                nc.scalar.dma_start(
                    out=wv_sb[:],
                    in_=wvT.ap().rearrange("(g p) o -> p g o", p=P))
                nc.scalar.dma_start(
                    out=wf_sb[:],
                    in_=wfT.ap().rearrange("(g p) o -> p g o", p=P))
                # ones columns of V_aug (cols 64:128 per head -> the PV matmul
                # replicates the softmax denominator over PSUM rows 64:128
                # for free, so no partition-broadcast is ever needed)
                # ones in cols 0:64 (NOT 64:128): the den replicas must land
                # at PSUM partition offset 0 -- custom-DVE ops (recip) ignore
                # nonzero input partition offsets on real hardware.
                for tt in range(2 * 4):
                    nc.gpsimd.memset(
                        vaug_sb.rearrange("p t (h c) -> p t h c",
                                          c=HA)[:, tt, :, 0:64],
                        1.0)
                # zero halves of the padded K stores (never written elsewhere)
                nc.gpsimd.memset(kTe_sb[64:128, :, :], 0.0)
                nc.gpsimd.memset(kTo_sb[0:64, :, :], 0.0)

                sec = (lambda name: sections is None or name in sections)

                # ---- Q^T / K^T projection block j ----
                # weight column-block j is loaded once; both slots'
                # accumulation chains are interleaved per g so consecutive
                # matmuls hit different PSUM banks.
                def qk_block(j):
                    for wT_d, b_sb, dst in ((wqT, bq_sb, qT_sb),
                                            (wkT, bk_sb, None)):
                        wblk = wqk_pool.tile([P, G, P], FQK, tag="wqk")
                        nc.sync.dma_start(out=wblk[:], in_=wT_d.ap()[j])
                        pss = {}
                        for s in range(B_LOC):
                            if ns[s]:
                                pss[s] = ps_mm.tile([P, 512], f32, tag="psmm", name=f"psqk{s}")
                        for g in range(G):
                            for s, ps in pss.items():
                                nc.tensor.matmul(
                                    ps[:, :ns[s] * P], lhsT=wblk[:, g, :],
                                    rhs=xT_sb[:, g, s * S:s * S + ns[s] * P],
                                    start=(g == 0), stop=(g == G - 1))
                        for s, ps in pss.items():
                            W = ns[s] * P
                            if dst is not None:  # Q
                                nc.vector.tensor_scalar_add(
                                    out=dst[:, j, s * S:s * S + W],
                                    in0=ps[:, :W], scalar1=b_sb[:, j:j + 1])
                            else:  # K -> split into the two padded stores
                                nc.vector.tensor_scalar_add(
                                    out=kTe_sb[0:64, j, s * S:s * S + W],
                                    in0=ps[0:64, :W],
                                    scalar1=b_sb[0:64, j:j + 1])
                                nc.vector.tensor_scalar_add(
                                    out=kTo_sb[64:128, j, s * S:s * S + W],
                                    in0=ps[64:128, :W],
                                    scalar1=b_sb[64:128, j:j + 1])

                # ---- V projection (natural layout, into V_aug) ----
                def v_all():
                    for ob in range(2):  # 512-wide output column blocks
                        for s in range(B_LOC):
                            for tt in range(ns[s]):
                                tg = s * 4 + tt
                                ps = ps_mm.tile([P, 512], f32, tag="psmm")
                                for g in range(G):
                                    nc.tensor.matmul(
                                        ps[:],
                                        lhsT=xT_sb[:, g, tg * P:(tg + 1) * P],
                                        rhs=wv_sb[:, g, ob * 512:(ob + 1) * 512],
                                        start=(g == 0), stop=(g == G - 1))
                                vdst = vaug_sb[:, tg, :].rearrange(
                                    "p (h c) -> p h c", c=HA)[:, ob * 8:(ob + 1) * 8, 64:128]
                                nc.vector.tensor_add(
                                    vdst,
                                    ps[:].rearrange("p (h c) -> p h c", c=64),
                                    bv_sb[:, ob * 512:(ob + 1) * 512].rearrange(
                                        "p (h c) -> p h c", c=64))

                # ---- attention for one head pair g2 of one slot ----
                def scores_exp(g2, s):
                    n = ns[s]
                    W = n * P
                    s0 = s * S
                    pts = {0: [], 1: []}
                    for kt in range(n):
                        for hh, kpad in ((0, kTe_sb), (1, kTo_sb)):
                            ps = ps_sc.tile([P, 512], f32, tag="pssc")
                            nc.tensor.matmul(
                                ps[:, :W],
                                lhsT=kpad[:, g2,
                                          s0 + kt * P:s0 + (kt + 1) * P],
                                rhs=qT_sb[:, g2, s0:s0 + W],
                                start=True, stop=True)
                            if not sec("attn_exp"):
                                continue
                            pt = pt_pool.tile([P, 512], FPV, tag="pt")
                            nc.scalar.activation(
                                out=pt[:, :W], in_=ps[:, :W],
                                func=mybir.ActivationFunctionType.Exp,
                                bias=kb_sb[:, s, kt:kt + 1], scale=0.125)
                            pts[hh].append(pt)
                    return pts

                def pv_tail(g2, s, pts):
                    n = ns[s]
                    W = n * P
                    s0 = s * S
                    # the two heads' PV accumulation chains interleaved so
                    # consecutive matmuls hit different PSUM banks
                    pvs = {hh: ps_pv.tile([P, 512], f32, tag="pspv",
                                          name=f"pv{hh}")
                           for hh in (0, 1)}
                    for kt in range(n):
                        for hh in (0, 1):
                            h = 2 * g2 + hh
                            nc.tensor.matmul(
                                pvs[hh][:, :W],
                                lhsT=vaug_sb[:, s * 4 + kt,
                                             HA * h:HA * h + HA],
                                rhs=pts[hh][kt][:, :W],
                                start=(kt == 0), stop=(kt == n - 1))
                    if not sec("attn_tail"):
                        return
                    for hh in (0, 1):
                        pv = pvs[hh]
                        po = hh * 64
                        # PSUM rows 0:64 all hold the softmax denominator
                        # (ones-columns of V_aug, which must sit at cols 0:64:
                        # custom-DVE ops ignore nonzero input partition
                        # offsets on real hardware); invert on DVE and
                        # normalize -- no cross-engine hop at all.
                        rb = rb_pool.tile([64, 512], f32, tag="rb")
                        nc.vector.reciprocal_approx_fast(
                            out=rb[:, :W], in_=pv[0:64, :W])
                        nc.vector.tensor_mul(
                            aoT_sb[po:po + 64, g2, s0:s0 + W],
                            pv[64:128, :W], rb[:, :W])

                def attn_g2(g2):
                    if not sec("attn"):
                        return
                    live = [s for s in slot_order if ns[s]]
                    pts = {s: scores_exp(g2, s) for s in live}
                    if sec("attn_pv"):
                        for s in live:
                            pv_tail(g2, s, pts[s])

                # ---- FC (per slot so the small slot's fc overlaps the
                # big slot's attention) ----
                def fc_slot(s):
                    for oh in range(2):
                        for tt in range(ns[s]):
                            tg = s * 4 + tt
                            ps = ps_mm.tile([P, 512], f32, tag="psmm")
                            for g in range(G):
                                nc.tensor.matmul(
                                    ps[:],
                                    lhsT=aoT_sb[:, g, tg * P:(tg + 1) * P],
                                    rhs=wf_sb[:, g, oh * 512:(oh + 1) * 512],
                                    start=(g == 0), stop=(g == G - 1))
                            ot = out_pool.tile([P, 512], f32, tag="out")
                            nc.vector.tensor_add(
                                ot[:], ps[:], bf_sb[:, oh * 512:(oh + 1) * 512])
                            nc.gpsimd.dma_start(
                                out=y.ap()[tg * P:(tg + 1) * P,
                                           oh * 512:(oh + 1) * 512],
                                in_=ot[:])

                # ---- j-pipelined emission: attention for head pair j-1
                # rides behind projection block j, so the ACT exps hide
                # under the next block's projection matmuls and the QK
                # drains of block j complete while attn(j-1) runs.
                slot_order = sorted(range(B_LOC), key=lambda s: ns[s])
                if sec("qk"):
                    qk_block(0)
                if sec("v"):
                    v_all()
                for j in range(1, G):
                    if sec("qk"):
                        qk_block(j)
                    attn_g2(j - 1)
                attn_g2(G - 1)
                if sec("fc"):
                    for s in slot_order:
                        if ns[s]:
                            fc_slot(s)

            if loop_iters is None:
                body()
            else:
                # benchmark-only loop; prefetch hints keep the back-edge
                # branch from paying an IRAM refetch on every iteration
                with tc.For_i(0, loop_iters, 1,
                              hint_engines=tuple(mybir.ALL_ENGINES)):
                    body()

    nc.compile()
    return nc


def _make_runner(nc):
    """Compile nc into a reusable 8-core jitted callable (axon PJRT path)."""
    import jax
    import numpy as _np
    from jax.experimental.shard_map import shard_map
    from jax.sharding import Mesh, NamedSharding, PartitionSpec

    import concourse.mybir as mybir
    from concourse import bass2jax

    bass2jax.install_neuronx_cc_hook()
    partition_name = (nc.partition_id_tensor.name
                      if nc.partition_id_tensor else None)
    in_names, out_names, out_avals, zero_outs = [], [], [], []
    for alloc in nc.m.functions[0].allocations:
        if not isinstance(alloc, mybir.MemoryLocationSet):
            continue
        name = alloc.memorylocations[0].name
        if alloc.kind == "ExternalInput":
            if name != partition_name:
                in_names.append(name)
        elif alloc.kind == "ExternalOutput":
            shape = tuple(alloc.tensor_shape)
            dtype = mybir.dt.np(alloc.dtype)
            out_names.append(name)
            out_avals.append(jax.core.ShapedArray(shape, dtype))
            zero_outs.append(_np.zeros(shape, dtype))
    n_params = len(in_names)
    in_names_all = in_names + out_names
    if partition_name is not None:
        in_names_all.append(partition_name)

    def _body(*args):
        operands = list(args)
        if partition_name is not None:
            operands.append(bass2jax.partition_id_tensor())
        outs = bass2jax._bass_exec_p.bind(
            *operands, out_avals=tuple(out_avals),
            in_names=tuple(in_names_all), out_names=tuple(out_names),
            lowering_input_output_aliases=(),
            sim_require_finite=True, sim_require_nnan=True, nc=nc)
        return tuple(outs)

    devices = jax.devices()[:N_CORES]
    mesh = Mesh(np.asarray(devices), ("core",))
    nio = n_params + len(out_names)
    sharded = jax.jit(
        shard_map(_body, mesh=mesh,
                  in_specs=(PartitionSpec("core"),) * nio,
                  out_specs=(PartitionSpec("core"),) * len(out_names),
                  check_rep=False),
        keep_unused=True)
    sharding = NamedSharding(mesh, PartitionSpec("core"))

    def stage(in_maps):
        per_core = [[_np.asarray(m[nm]) for nm in in_names] for m in in_maps]
        concat_in = [
            _np.concatenate([per_core[c][i] for c in range(N_CORES)], axis=0)
            for i in range(n_params)
        ]
        concat_zeros = [
            _np.zeros((N_CORES * z.shape[0], *z.shape[1:]), z.dtype)
            for z in zero_outs
        ]
        dev_in = [jax.device_put(a, sharding)
                  for a in concat_in + concat_zeros]
        jax.block_until_ready(dev_in)
        return dev_in

    def execute(dev_in):
        out = sharded(*dev_in)
        jax.block_until_ready(out)
        return out

    def fetch(out):
        return [
            {nm: _np.asarray(out[i]).reshape(N_CORES, *out_avals[i].shape)[c]
             for i, nm in enumerate(out_names)}
            for c in range(N_CORES)
        ]

    def run(in_maps):
        return fetch(execute(stage(in_maps)))

    run.stage = stage
    run.execute = execute
    run.fetch = fetch
    return run


def _prep(lstm_output, cls_len, wq, bq, wk, bk, wv, bv, wf, bf, qk_np, fc_np):
    """Host-side prep: sample->slot assignment + per-core input maps."""
    x = np.asarray(lstm_output, dtype=np.float32)
    cls = np.asarray(cls_len).astype(np.int64)
    order = np.argsort(-cls, kind="stable")
    slots = [order[:N_CORES], order[N_CORES:]]
    ns = tuple(
        int(math.ceil(int(cls[sl].max()) / P)) if len(sl) else 0
        for sl in slots)

    def _swz(w, npdt):
        # w [o, i] -> w.T [i, o] -> [j, p, g, c]: block j holds output cols
        # j*128..(j+1)*128 for all 8 input chunks, partition-major
        wt = np.asarray(w, np.float32).T.reshape(G, P, G, P)  # [g, p, j, c]
        return np.ascontiguousarray(wt.transpose(2, 1, 0, 3)).astype(npdt)

    wqT = _swz(wq, qk_np)
    wkT = _swz(wk, qk_np)
    wvT = np.asarray(wv, np.float32).T.astype(qk_np)
    wfT = np.asarray(wf, np.float32).T.astype(fc_np)
    bq2 = np.asarray(bq, np.float32).reshape(G, P)
    bk2 = np.asarray(bk, np.float32).reshape(G, P)
    bv1 = np.asarray(bv, np.float32).reshape(1, H)
    bf1 = np.asarray(bf, np.float32).reshape(1, H)

    idx = np.arange(S)
    in_maps = []
    assign = []  # (core, slot) -> sample
    for c in range(N_CORES):
        samples = [int(slots[0][c]), int(slots[1][c])]
        assign.append(samples)
        xc = np.concatenate([x[b] for b in samples], axis=0)  # [T, H]
        xTc = np.ascontiguousarray(xc.T).astype(qk_np)  # [H, T]
        kb = np.zeros((B_LOC, 4, P), np.float32)
        for s, b in enumerate(samples):
            L = int(cls[b])
            kb[s] = np.where(idx < L, 0.0, NEG).reshape(4, P)
        in_maps.append({
            "xT": xTc, "wqT": wqT, "wkT": wkT, "wvT": wvT, "wfT": wfT,
            "bq2": bq2, "bk2": bk2, "bv1": bv1, "bf1": bf1,
            "kbias": kb,
        })
    return in_maps, assign, ns, cls, np.asarray(bf, np.float32)


def _gather(results, assign, cls, bf):
    out = np.empty((B, S, H), np.float32)
    for c in range(N_CORES):
        yc = results[c]["y"]  # [T, H]
        for s, b in enumerate(assign[c]):
            out[b] = yc[s * S:(s + 1) * S]
            L = int(cls[b])
            out[b, L:, :] = bf  # rows at/after cls_len are exactly the fc bias
    return out


def kernel(lstm_output, cls_len, wq, bq, wk, bk, wv, bv, wf, bf):
    import ml_dtypes
    qk_np = ml_dtypes.bfloat16
    fc_np = ml_dtypes.bfloat16

    in_maps, assign, ns, cls, bf_np = _prep(
        lstm_output, cls_len, wq, bq, wk, bk, wv, bv, wf, bf, qk_np, fc_np)

    key = ("run", ns)
    if key not in _CACHE:
        nc = _build(ns)
        _CACHE[key] = _make_runner(nc)
    run = _CACHE[key]
    results = run(in_maps)
    return _gather(results, assign, cls, bf_np)

